# revision 73
# baseline (speedup 1.0000x reference)
"""Trainium2 Bass kernel for the CapsuleLayer routing problem.

~104-125us steady-state on silicon, median ~110-120 depending on fabric
noise (from ~142-180us for the 3-collective f32r baseline in the same
conditions), rel err 6.4e-3 vs the fp32 reference (harness gate 2e-2).

Strategy (i-sharded routing with a replicated, collective-free iter 0):
  - Iteration 0's coupling coefficients are uniform (c = 1/10), so
    s0 = 0.1 * x @ W needs no routing state. Every core computes the FULL
    s0 redundantly from replicated bf16 copies of x^T and W (144 matmuls)
    - that work hides inside the ~45-60us window where the collective
    stream's startup barrier + first-op setup block any collective anyway.
    This removes one AllReduce from the critical path: 2 collectives left.
  - The replicated tensors are tile-ROTATED per core so each core's own
    i-shard sits at tiles [0..NT): shard matmuls (s1/s2, P, z, Wc) slice
    the replicated tiles directly - no separate shard loads, no dynamic
    offsets, and the s0 sum is order-invariant.
  - b-logit updates are per-i-local; s1 = x_shard @ (c1 ⊙ W_shard) is a
    partial sum over i, combined with ONE bf16 AllReduce. The final
    iteration's AllReduce is replaced by a ReduceScatter; each core
    squashes its own 32 batches and the host reorders slices.
  - A tiny warmup AllReduce is issued first: the CC stream's ~11us
    first-op setup + cold-fabric cost burns on it during the s0 window,
    so the real AllReduce runs warm (~11us instead of ~27us). With the
    iter-0 collective gone this is pure win (measured; it was neutral
    when a real collective fired early anyway).
  - u_hat (189 MB) is never materialized anywhere.

Perf notes (from ntff profiles on silicon):
  - Collective cost is ~100% fixed overhead at this size: a 512B
    AllReduce takes 11us, 160KB bf16 takes ~11us warm (RDH, ~1.5us/step).
  - Everything computes in bf16 matmuls (f32 PSUM accumulation is exact;
    measured 2.5e-7 per matmul): bf16 @ 160 moving cols streams ~2x
    faster than f32r @ 256-padded and halves the input DMA (8.2MB/core,
    ~400KB/us aggregate HBM rate).
  - PSUM accumulation chains may interleave ONLY across different 2KB
    banks (same-bank interleave corrupts accumulation ~11%; separate
    banks measured exact). s0 and the shard s-matmuls interleave their
    two batch-half chains tile-by-tile in separate banks so inputs are
    consumed on arrival and both halves stage together.
  - pp PSUM tiles stay 256-wide so b-update chunks align to 2KB banks;
    vector z reads never touch banks in-flight matmuls write. Chunk
    boundaries must be even tile counts; (2,2,2,3) beat (4,4,1) by ~5us
    (shorter critical prefix before the next s-matmul starts). Pad
    columns are never written or read.
  - All host tensors are [128, X] partition-major: contiguous DMA runs.
  - Only Sqrt/Exp run on the scalar engine; Exp reads the k-sum matmul's
    PSUM directly via a running product of exp(y) ("b" never
    materialized).
"""
import sys

for _p in ("/opt/trn_rl_repo",):
    if _p not in sys.path:
        sys.path.insert(0, _p)

import numpy as np

import concourse.bass as bass
import concourse.bacc as bacc
import concourse.mybir as mybir
import concourse.tile as tile
from concourse.bass_utils import run_bass_kernel_spmd

F32 = mybir.dt.float32
BF16 = mybir.dt.bfloat16
AF = mybir.ActivationFunctionType
ALU = mybir.AluOpType

IN_NODES, OUT_NODES = 1152, 10
IN_DIM, OUT_DIM = 8, 16
B = 256
N_CORES = 8
I_LOC = IN_NODES // N_CORES          # 144
IK = I_LOC * IN_DIM                  # 1152
NT = IK // 128                       # 9 tiles over the shard (i,k) axis
NTF = IN_NODES * IN_DIM // 128       # 72 tiles over the full (i,k) axis
JD = OUT_NODES * OUT_DIM             # 160
JDP = 256                            # PSUM bank-align pad (pp tiles only)
B_LOC = B // N_CORES                 # 32 batches per core after RS
RG = [list(range(N_CORES))]


def build_nc(repeat=1):
    nc = bacc.Bacc(
        "TRN2",
        target_bir_lowering=False,
        debug=False,
        enable_asserts=False,
        num_devices=N_CORES,
    )
    # full tensors, tile-rotated per core (own shard first), partition-major
    xTF_d = nc.dram_tensor("xTF", [128, NTF * B], BF16, kind="ExternalInput")
    wf_d = nc.dram_tensor("wf", [128, NTF * JD], BF16, kind="ExternalInput")
    xb_d = nc.dram_tensor("xb", [128, 2 * IK], BF16, kind="ExternalInput")
    # 1/B = 1/256 is exact in bf16; bf16 lets the y k-sum matmul take the
    # bf16 y_all directly
    ones_d = nc.dram_tensor("onesb", [128, 128], BF16, kind="ExternalInput")
    out_d = nc.dram_tensor("out", [B_LOC, JD], F32, kind="ExternalOutput")

    with tile.TileContext(nc) as tc:
        with (
            tc.tile_pool(name="big", bufs=1) as bigp,
            tc.tile_pool(name="work", bufs=2) as workp,
            tc.tile_pool(name="psum", bufs=2, space="PSUM") as psum,
            tc.tile_pool(name="dram", bufs=2, space="DRAM") as dramp,
        ):
            xTF_sb = bigp.tile([128, NTF * B], BF16)      # 36.9KB/part
            WF_sb = bigp.tile([128, NTF, JD], BF16)       # 23KB/part
            Wc_sb = bigp.tile([128, NT, JD], BF16)
            x_sb = bigp.tile([128, 2 * IK], BF16)
            v_sb = bigp.tile([128, 2, JD], BF16)
            ones_sb = bigp.tile([128, 128], BF16)
            e_acc = bigp.tile([128, NT * OUT_NODES], F32)  # running exp(b)

            for _rep in range(repeat):
                # tiny warmup collective first: the CC stream's ~11us
                # first-op setup + cold-fabric cost burn here, hidden under
                # s0 (input is uninitialized scratch; the summed garbage is
                # never read). A full-size 80KB warmup measured WORSE: it
                # runs ~15us, delays the real AllReduce's slot, and the
                # real op still ran 19-21us.
                win_t = dramp.tile([1, 128], F32, tag="wcc_in")
                wout = dramp.tile([1, 128], F32, tag="wcc_out",
                                  addr_space="Shared")
                nc.gpsimd.collective_compute(
                    "AllReduce", ALU.add, replica_groups=RG,
                    ins=[win_t[:]], outs=[wout[:]],
                )
                nc.gpsimd.dma_start(ones_sb[:], ones_d[:])
                # full bf16 tensors chunked over 12 x 6 tiles across 3
                # engine queues so the s0 matmuls start after ~1/12 of the
                # bytes and the arrival pacing is fine-grained
                dma_engs = [nc.sync, nc.scalar, nc.gpsimd]
                wf_v = WF_sb[:].rearrange("p t x -> p (t x)")
                for ch in range(12):
                    e = dma_engs[ch % 3]
                    e.dma_start(wf_v[:, 6 * ch * JD:6 * (ch + 1) * JD],
                                wf_d[:, 6 * ch * JD:6 * (ch + 1) * JD])
                    e.dma_start(xTF_sb[:, 6 * ch * B:6 * (ch + 1) * B],
                                xTF_d[:, 6 * ch * B:6 * (ch + 1) * B])
                nc.sync.dma_start(x_sb[:], xb_d[:])
                nc.gpsimd.memset(e_acc[:], 1.0)
                # prime both ACT tables (Sqrt/Exp) off the critical path
                tprime = workp.tile([128, 8], F32, tag="tprime")
                nc.scalar.activation(tprime[:], ones_sb[:, 0:8], AF.Sqrt)
                nc.scalar.activation(tprime[:], ones_sb[:, 0:8], AF.Exp)

                def squash_pb_update(s_src, cs, lowp=False):
                    """v = squash(s_src) -> P = x^T v -> y -> softmax -> Wc.

                    s_src: AP [128, 2, JD] (SBUF f32 for iter 0, bf16 after
                    the AllReduce). cs folds the uniform iter-0 c:
                    f = cs*sqrt(sq)/(1+cs*sq), via rt = sqrt(cs^2*sq).
                    lowp runs the big ssq/sq ops in bf16 2x DVE mode (only
                    used when v feeds routing state, never the output).
                    """
                    sdt = BF16 if lowp else F32
                    ssq = workp.tile([128, 2, JD], sdt, tag="ssq")
                    nc.vector.tensor_tensor(ssq[:], s_src, s_src,
                                            op=ALU.mult)
                    sq = workp.tile([128, 2 * OUT_NODES], sdt, tag="sq")
                    with nc.allow_low_precision(
                            reason="v1 feeds routing state only"):
                        nc.vector.tensor_reduce(
                            sq[:],
                            ssq[:].rearrange("p g (j d) -> p g j d",
                                             d=OUT_DIM),
                            axis=mybir.AxisListType.X, op=ALU.add,
                        )
                    rt = workp.tile([128, 2 * OUT_NODES], F32, tag="rt")
                    nc.scalar.activation(rt[:], sq[:], AF.Sqrt,
                                         scale=cs * cs)
                    den = workp.tile([128, 2 * OUT_NODES], F32, tag="den")
                    nc.vector.tensor_scalar(den[:], sq[:], cs, 1.0,
                                            ALU.mult, ALU.add)
                    dri = workp.tile([128, 2 * OUT_NODES], F32, tag="dri")
                    nc.vector.reciprocal(dri[:], den[:])
                    f = workp.tile([128, 2 * OUT_NODES], F32, tag="f")
                    nc.vector.tensor_tensor(f[:], rt[:], dri[:],
                                            op=ALU.mult)
                    f_b = (f[:].rearrange("p (g j) -> p g j", j=OUT_NODES)
                           .unsqueeze(3)
                           .broadcast_to([128, 2, OUT_NODES, OUT_DIM]))
                    nc.vector.tensor_tensor(
                        v_sb[:].rearrange("p g (j d) -> p g j d", d=OUT_DIM),
                        s_src.rearrange("p g (j d) -> p g j d", d=OUT_DIM),
                        f_b, op=ALU.mult,
                    )
                    # ---- P = x^T @ v ; Y = reduce_d(W ⊙ P) ; k-sum ;
                    # e_acc *= exp(y) ; c = softmax ; Wc = W ⊙ c ----
                    # The softmax over j is per-(i,k)-row LOCAL, so the
                    # WHOLE b-update chain pipelines per tile-chunk: the
                    # next s-matmul starts after chunk 0's Wc instead of
                    # after the full update. Chunks [0:4)[4:8)[8:9) align
                    # to 2KB PSUM banks (z reads never touch banks
                    # in-flight matmuls write). z/y in bf16 2x DVE mode;
                    # y only feeds routing logits. b logits are never
                    # materialized: e_acc carries the running exp, and Exp
                    # reads the k-sum matmul's PSUM directly.
                    pp_ps = psum.tile([128, NT, JDP], F32, tag="pp_ps",
                                      bufs=1)
                    z_all = workp.tile([128, NT, JD], BF16, tag="z_all")
                    y_all = workp.tile([128, NT * OUT_NODES], BF16,
                                       tag="y_all")
                    y_ps = psum.tile([128, NT * OUT_NODES], F32,
                                     tag="y_ps", bufs=1)
                    ey = workp.tile([128, NT * OUT_NODES], F32, tag="ey")
                    dsum = workp.tile([128, NT], F32, tag="dsum")
                    r = workp.tile([128, NT], F32, tag="r")
                    # c in bf16 so the Wc TTs run all-16-bit in 2x DVE mode
                    c = workp.tile([128, NT * OUT_NODES], BF16, tag="c")
                    r_b = r[:].unsqueeze(2).broadcast_to(
                        [128, NT, OUT_NODES])
                    c_b = (c[:].rearrange("p (t j) -> p t j", j=OUT_NODES)
                           .unsqueeze(3)
                           .broadcast_to([128, NT, OUT_NODES, OUT_DIM]))
                    for lo, hi in ((0, 2), (2, 4), (4, 6), (6, 9)):
                        for t in range(lo, hi):
                            for b0 in range(2):
                                nc.tensor.matmul(
                                    pp_ps[:, t, 0:JD],
                                    x_sb[:, b0 * IK + t * 128:
                                         b0 * IK + t * 128 + 128],
                                    v_sb[:, b0, :],
                                    start=(b0 == 0),
                                    stop=(b0 == 1),
                                )
                        slc = slice(lo, hi)
                        cols = slice(lo * OUT_NODES, hi * OUT_NODES)
                        nc.vector.tensor_tensor(
                            z_all[:, slc, :], WF_sb[:, slc, :],
                            pp_ps[:, slc, 0:JD], op=ALU.mult,
                        )
                        with nc.allow_low_precision(
                                reason="y feeds routing logits only; "
                                       "bf16 d-sum of 16 terms is ample"):
                            nc.vector.tensor_reduce(
                                y_all[:].rearrange("p (t j) -> p t j",
                                                   j=OUT_NODES)[:, slc, :],
                                z_all[:, slc, :].rearrange(
                                    "p t (j d) -> p t j d", d=OUT_DIM),
                                axis=mybir.AxisListType.X, op=ALU.add,
                            )
                        # ones_sb holds 1/B in 8x8 diag blocks: k-sum,
                        # broadcast back over k, and the 1/B in one matmul
                        nc.tensor.matmul(y_ps[:, cols], ones_sb[:],
                                         y_all[:, cols],
                                         start=True, stop=True)
                        nc.scalar.activation(ey[:, cols], y_ps[:, cols],
                                             AF.Exp)
                        nc.vector.tensor_tensor(e_acc[:, cols],
                                                e_acc[:, cols],
                                                ey[:, cols], op=ALU.mult)
                        nc.vector.tensor_reduce(
                            dsum[:, lo:hi],
                            e_acc[:].rearrange("p (t j) -> p t j",
                                               j=OUT_NODES)[:, slc, :],
                            axis=mybir.AxisListType.X, op=ALU.add,
                        )
                        nc.vector.reciprocal(r[:, lo:hi], dsum[:, lo:hi])
                        nc.vector.tensor_tensor(
                            c[:].rearrange("p (t j) -> p t j",
                                           j=OUT_NODES)[:, slc, :],
                            e_acc[:].rearrange("p (t j) -> p t j",
                                               j=OUT_NODES)[:, slc, :],
                            r_b[:, slc], op=ALU.mult,
                        )
                        nc.vector.tensor_tensor(
                            Wc_sb[:, slc, :].rearrange(
                                "p t (j d) -> p t j d", d=OUT_DIM),
                            WF_sb[:, slc, :].rearrange(
                                "p t (j d) -> p t j d", d=OUT_DIM),
                            c_b[:, slc], op=ALU.mult,
                        )

                # ================= iteration 0 (collective-free) ==========
                # s0 = x_full @ W_full computed redundantly on every core
                # from the replicated bf16 tensors; c folding via cs = 0.01.
                # The two batch-half chains are INTERLEAVED tile-by-tile so
                # each xTF/WF tile is consumed for both halves the moment
                # its DMA chunk lands (s0 is input-arrival-paced). That is
                # only legal because the chains sit in SEPARATE PSUM banks
                # (pp tiles 0 and 4 = banks 0 and 2): interleaving two
                # start/stop chains in ONE bank corrupts the accumulation
                # (measured 11% error; separate banks measured exact).
                s0_ps = psum.tile([128, NT, JDP], F32, tag="pp_ps", bufs=1)
                for T in range(NTF):
                    for b0 in range(2):
                        nc.tensor.matmul(
                            s0_ps[:, 4 * b0, 0:JD],
                            xTF_sb[:, T * B + b0 * 128:
                                   T * B + b0 * 128 + 128],
                            WF_sb[:, T, :],
                            start=(T == 0),
                            stop=(T == NTF - 1),
                        )
                # tensor_tensor may read only ONE input from PSUM: copy out
                # cast to bf16 on the way out so iter 0's squash also runs
                # its big ops in 2x DVE mode (v0 is routing-state only)
                s0_sb = workp.tile([128, 2, JD], BF16, tag="s0_sb")
                nc.vector.tensor_copy(s0_sb[:, 0], s0_ps[:, 0, 0:JD])
                nc.vector.tensor_copy(s0_sb[:, 1], s0_ps[:, 4, 0:JD])
                squash_pb_update(s0_sb[:], 0.01, lowp=True)

                # ================= iterations 1..2 ========================
                for it in (1, 2):
                    # s-matmul on the own shard (= rotated tiles [0..NT)):
                    # partial over local i. The two batch-half chains are
                    # interleaved tile-by-tile in SEPARATE PSUM banks
                    # ([2, 512] f32 = one bank per half) so each Wc tile
                    # feeds both halves as soon as its chunk is ready and
                    # both halves finish together - no second 9-matmul
                    # pass on the staging critical path.
                    s_ps = psum.tile([128, 2, 512], F32, tag="s_ps", bufs=1)
                    s_stage = workp.tile([128, 2, JD], BF16, tag="s_stage")
                    sin = dramp.tile([2, 128, JD], BF16, tag="cc_in")
                    for t in range(NT):
                        for b0 in range(2):
                            nc.tensor.matmul(
                                s_ps[:, b0, 0:JD],
                                xTF_sb[:, t * B + b0 * 128:
                                       t * B + b0 * 128 + 128],
                                Wc_sb[:, t, :],
                                start=(t == 0),
                                stop=(t == NT - 1),
                            )
                    for b0 in range(2):
                        nc.vector.tensor_copy(s_stage[:, b0],
                                              s_ps[:, b0, 0:JD])
                        # half-1 stage DMA on gpsimd: same queue as the
                        # collective trigger, avoids a cross-engine hop
                        (nc.sync if b0 == 0 else nc.gpsimd).dma_start(
                            sin[b0], s_stage[:, b0])

                    if it == 1:
                        sout = dramp.tile([2, 128, JD], BF16, tag="cc_out",
                                          addr_space="Shared")
                        nc.gpsimd.collective_compute(
                            "AllReduce", ALU.add, replica_groups=RG,
                            ins=[sin[:]], outs=[sout[:]],
                        )
                        s_sb = workp.tile([128, 2, JD], BF16, tag="s_sb")
                        nc.sync.dma_start(s_sb[:, 0, :], sout[0])
                        nc.scalar.dma_start(s_sb[:, 1, :], sout[1])
                        squash_pb_update(s_sb[:], 1.0, lowp=True)
                    else:
                        # final iter: ReduceScatter; sin flat order is
                        # (g, p, j) so core c owns the 32 consecutive
                        # batches 128*(c//4) + 32*(c%4) + [0,32)
                        sout_rs = dramp.tile([B_LOC, JD], BF16,
                                             tag="cc_out_rs")
                        nc.gpsimd.collective_compute(
                            "ReduceScatter", ALU.add, replica_groups=RG,
                            ins=[sin[:]], outs=[sout_rs[:]],
                        )
                        sl = workp.tile([B_LOC, JD], BF16, tag="sl")
                        nc.sync.dma_start(sl[:], sout_rs[:])
                        ssq_l = workp.tile([B_LOC, JD], F32, tag="ssq_l")
                        nc.vector.tensor_tensor(ssq_l[:], sl[:], sl[:],
                                                op=ALU.mult)
                        sq_l = workp.tile([B_LOC, OUT_NODES], F32,
                                          tag="sq_l")
                        nc.vector.tensor_reduce(
                            sq_l[:],
                            ssq_l[:].rearrange("p (j d) -> p j d",
                                               d=OUT_DIM),
                            axis=mybir.AxisListType.X, op=ALU.add,
                        )
                        rt_l = workp.tile([B_LOC, OUT_NODES], F32,
                                          tag="rt_l")
                        nc.scalar.activation(rt_l[:], sq_l[:], AF.Sqrt)
                        den_l = workp.tile([B_LOC, OUT_NODES], F32,
                                           tag="den_l")
                        nc.vector.tensor_scalar_add(den_l[:], sq_l[:], 1.0)
                        dri_l = workp.tile([B_LOC, OUT_NODES], F32,
                                           tag="dri_l")
                        nc.vector.reciprocal(dri_l[:], den_l[:])
                        f_l = workp.tile([B_LOC, OUT_NODES], F32,
                                         tag="f_l")
                        nc.vector.tensor_tensor(f_l[:], rt_l[:], dri_l[:],
                                                op=ALU.mult)
                        v_l = workp.tile([B_LOC, JD], F32, tag="v_l")
                        f_lb = (f_l[:].unsqueeze(2)
                                .broadcast_to([B_LOC, OUT_NODES, OUT_DIM]))
                        nc.vector.tensor_tensor(
                            v_l[:].rearrange("p (j d) -> p j d", d=OUT_DIM),
                            sl[:].rearrange("p (j d) -> p j d", d=OUT_DIM),
                            f_lb, op=ALU.mult,
                        )
                        nc.sync.dma_start(out_d[:], v_l[:])

    nc.compile()
    return nc


def make_inmaps(x, W):
    import ml_dtypes
    bf16 = ml_dtypes.bfloat16
    x = np.ascontiguousarray(np.asarray(x, dtype=np.float32))
    W = np.ascontiguousarray(np.asarray(W, dtype=np.float32))
    x_all = x.reshape(B, IN_NODES * IN_DIM)
    # full tensors as [128, NTF, ...] tile-major views (global tile order)
    xTF_g = np.ascontiguousarray(
        x_all.T.reshape(NTF, 128, B).transpose(1, 0, 2))      # [128,NTF,B]
    wf_g = np.ascontiguousarray(
        W.transpose(0, 3, 1, 2).reshape(NTF, 128, JD).transpose(1, 0, 2))
    # 16 8x8 blocks of 1/B on the diagonal (1/256 is exact in bf16)
    ones_blk = (np.kron(np.eye(128 // IN_DIM, dtype=np.float32),
                        np.ones((IN_DIM, IN_DIM), dtype=np.float32))
                / B).astype(bf16)
    in_maps = []
    for cid in range(N_CORES):
        # rotate tiles so the core's own shard sits at tiles [0..NT)
        rot = np.roll(np.arange(NTF), -cid * NT)
        xTF = np.ascontiguousarray(
            xTF_g[:, rot, :].reshape(128, NTF * B)).astype(bf16)
        wf = np.ascontiguousarray(
            wf_g[:, rot, :].reshape(128, NTF * JD)).astype(bf16)
        sh = slice(cid * I_LOC, (cid + 1) * I_LOC)
        x_sh = x[:, sh, :].reshape(B, IK)
        xb = np.ascontiguousarray(
            x_sh.reshape(2, 128, IK).transpose(1, 0, 2).reshape(
                128, 2 * IK)).astype(bf16)
        in_maps.append({
            "xTF": xTF, "wf": wf, "xb": xb, "onesb": ones_blk,
        })
    return in_maps


def assemble_output(per_core_outs):
    v = np.empty((B, OUT_NODES, OUT_DIM), dtype=np.float32)
    for c in range(N_CORES):
        o = per_core_outs[c]["out"].reshape(B_LOC, OUT_NODES, OUT_DIM)
        st = 128 * (c // 4) + B_LOC * (c % 4)
        v[st:st + B_LOC] = o
    return v[..., None].astype(np.float32)      # (256, 10, 16, 1)


_CACHED_NC = None


def kernel(x=None, W=None, **kw):
    global _CACHED_NC
    if x is None:
        x = kw["x"]
    if W is None:
        W = kw["W"]
    if _CACHED_NC is None:
        _CACHED_NC = build_nc()
    in_maps = make_inmaps(x, W)
    res = run_bass_kernel_spmd(
        _CACHED_NC, in_maps, core_ids=list(range(N_CORES)))
    return assemble_output(res.results)


if __name__ == "__main__":
    nc = build_nc()
    print("build + compile OK")


# revision 74
# speedup vs baseline: 1.2102x; 1.2102x over previous
"""Trainium2 Bass kernel for the CapsuleLayer routing problem.

~104-125us steady-state on silicon, median ~110-120 depending on fabric
noise (from ~142-180us for the 3-collective f32r baseline in the same
conditions), rel err 6.4e-3 vs the fp32 reference (harness gate 2e-2).

Strategy (i-sharded routing with a replicated, collective-free iter 0):
  - Iteration 0's coupling coefficients are uniform (c = 1/10), so
    s0 = 0.1 * x @ W needs no routing state. Every core computes the FULL
    s0 redundantly from replicated bf16 copies of x^T and W (144 matmuls)
    - that work hides inside the ~45-60us window where the collective
    stream's startup barrier + first-op setup block any collective anyway.
    This removes one AllReduce from the critical path: 2 collectives left.
  - The replicated tensors are tile-ROTATED per core so each core's own
    i-shard sits at tiles [0..NT): shard matmuls (s1/s2, P, z, Wc) slice
    the replicated tiles directly - no separate shard loads, no dynamic
    offsets, and the s0 sum is order-invariant.
  - b-logit updates are per-i-local; s1 = x_shard @ (c1 ⊙ W_shard) is a
    partial sum over i, combined with ONE bf16 AllReduce. The final
    iteration's AllReduce is replaced by a ReduceScatter; each core
    squashes its own 32 batches and the host reorders slices.
  - A tiny warmup AllReduce is issued first: the CC stream's ~11us
    first-op setup + cold-fabric cost burns on it during the s0 window,
    so the real AllReduce runs warm (~11us instead of ~27us). With the
    iter-0 collective gone this is pure win (measured; it was neutral
    when a real collective fired early anyway).
  - u_hat (189 MB) is never materialized anywhere.

Perf notes (from ntff profiles on silicon):
  - Collective cost is ~100% fixed overhead at this size: a 512B
    AllReduce takes 11us, 160KB bf16 takes ~11us warm (RDH, ~1.5us/step).
  - Everything computes in bf16 matmuls (f32 PSUM accumulation is exact;
    measured 2.5e-7 per matmul): bf16 @ 160 moving cols streams ~2x
    faster than f32r @ 256-padded and halves the input DMA (8.2MB/core,
    ~400KB/us aggregate HBM rate).
  - PSUM accumulation chains may interleave ONLY across different 2KB
    banks (same-bank interleave corrupts accumulation ~11%; separate
    banks measured exact). s0 and the shard s-matmuls interleave their
    two batch-half chains tile-by-tile in separate banks so inputs are
    consumed on arrival and both halves stage together.
  - pp PSUM tiles stay 256-wide so b-update chunks align to 2KB banks;
    vector z reads never touch banks in-flight matmuls write. Chunk
    boundaries must be even tile counts; (2,2,2,3) beat (4,4,1) by ~5us
    (shorter critical prefix before the next s-matmul starts). Pad
    columns are never written or read.
  - All host tensors are [128, X] partition-major: contiguous DMA runs.
  - Only Sqrt/Exp run on the scalar engine; Exp reads the k-sum matmul's
    PSUM directly via a running product of exp(y) ("b" never
    materialized).
"""
import sys

for _p in ("/opt/trn_rl_repo",):
    if _p not in sys.path:
        sys.path.insert(0, _p)

import numpy as np

import concourse.bass as bass
import concourse.bacc as bacc
import concourse.mybir as mybir
import concourse.tile as tile
from concourse.bass_utils import run_bass_kernel_spmd

F32 = mybir.dt.float32
BF16 = mybir.dt.bfloat16
AF = mybir.ActivationFunctionType
ALU = mybir.AluOpType

IN_NODES, OUT_NODES = 1152, 10
IN_DIM, OUT_DIM = 8, 16
B = 256
N_CORES = 8
I_LOC = IN_NODES // N_CORES          # 144
IK = I_LOC * IN_DIM                  # 1152
NT = IK // 128                       # 9 tiles over the shard (i,k) axis
NTF = IN_NODES * IN_DIM // 128       # 72 tiles over the full (i,k) axis
JD = OUT_NODES * OUT_DIM             # 160
JDP = 256                            # PSUM bank-align pad (pp tiles only)
B_LOC = B // N_CORES                 # 32 batches per core after RS
RG = [list(range(N_CORES))]


def build_nc(repeat=1):
    nc = bacc.Bacc(
        "TRN2",
        target_bir_lowering=False,
        debug=False,
        enable_asserts=False,
        num_devices=N_CORES,
    )
    # full tensors, tile-rotated per core (own shard first), partition-major
    xTF_d = nc.dram_tensor("xTF", [128, NTF * B], BF16, kind="ExternalInput")
    wf_d = nc.dram_tensor("wf", [128, NTF * JD], BF16, kind="ExternalInput")
    xb_d = nc.dram_tensor("xb", [128, 2 * IK], BF16, kind="ExternalInput")
    # 1/B = 1/256 is exact in bf16; bf16 lets the y k-sum matmul take the
    # bf16 y_all directly
    ones_d = nc.dram_tensor("onesb", [128, 128], BF16, kind="ExternalInput")
    out_d = nc.dram_tensor("out", [B_LOC, JD], F32, kind="ExternalOutput")

    with tile.TileContext(nc) as tc:
        with (
            tc.tile_pool(name="big", bufs=1) as bigp,
            tc.tile_pool(name="work", bufs=2) as workp,
            tc.tile_pool(name="psum", bufs=2, space="PSUM") as psum,
            tc.tile_pool(name="dram", bufs=2, space="DRAM") as dramp,
        ):
            xTF_sb = bigp.tile([128, NTF * B], BF16)      # 36.9KB/part
            WF_sb = bigp.tile([128, NTF, JD], BF16)       # 23KB/part
            Wc_sb = bigp.tile([128, NT, JD], BF16)
            x_sb = bigp.tile([128, 2 * IK], BF16)
            v_sb = bigp.tile([128, 2, JD], BF16)
            ones_sb = bigp.tile([128, 128], BF16)
            e_acc = bigp.tile([128, NT * OUT_NODES], F32)  # running exp(b)

            for _rep in range(repeat):
                # tiny warmup collective first: the CC stream's ~11us
                # first-op setup + cold-fabric cost burn here, hidden under
                # s0 (input is uninitialized scratch; the summed garbage is
                # never read). A full-size 80KB warmup measured WORSE: it
                # runs ~15us, delays the real AllReduce's slot, and the
                # real op still ran 19-21us.
                win_t = dramp.tile([1, 128], F32, tag="wcc_in")
                wout = dramp.tile([1, 128], F32, tag="wcc_out",
                                  addr_space="Shared")
                nc.gpsimd.collective_compute(
                    "AllReduce", ALU.add, replica_groups=RG,
                    ins=[win_t[:]], outs=[wout[:]],
                )
                nc.gpsimd.dma_start(ones_sb[:], ones_d[:])
                # full bf16 tensors chunked over 12 x 6 tiles across 3
                # engine queues so the s0 matmuls start after ~1/12 of the
                # bytes and the arrival pacing is fine-grained
                dma_engs = [nc.sync, nc.scalar, nc.gpsimd]
                wf_v = WF_sb[:].rearrange("p t x -> p (t x)")
                for ch in range(12):
                    e = dma_engs[ch % 3]
                    e.dma_start(wf_v[:, 6 * ch * JD:6 * (ch + 1) * JD],
                                wf_d[:, 6 * ch * JD:6 * (ch + 1) * JD])
                    e.dma_start(xTF_sb[:, 6 * ch * B:6 * (ch + 1) * B],
                                xTF_d[:, 6 * ch * B:6 * (ch + 1) * B])
                nc.sync.dma_start(x_sb[:], xb_d[:])
                nc.gpsimd.memset(e_acc[:], 1.0)
                # prime both ACT tables (Sqrt/Exp) off the critical path
                tprime = workp.tile([128, 8], F32, tag="tprime")
                nc.scalar.activation(tprime[:], ones_sb[:, 0:8], AF.Sqrt)
                nc.scalar.activation(tprime[:], ones_sb[:, 0:8], AF.Exp)

                def squash_pb_update(s_src, cs, lowp=False):
                    """v = squash(s_src) -> P = x^T v -> y -> softmax -> Wc.

                    s_src: AP [128, 2, JD] (SBUF f32 for iter 0, bf16 after
                    the AllReduce). cs folds the uniform iter-0 c:
                    f = cs*sqrt(sq)/(1+cs*sq), via rt = sqrt(cs^2*sq).
                    lowp runs the big ssq/sq ops in bf16 2x DVE mode (only
                    used when v feeds routing state, never the output).
                    """
                    sdt = BF16 if lowp else F32
                    ssq = workp.tile([128, 2, JD], sdt, tag="ssq")
                    nc.vector.tensor_tensor(ssq[:], s_src, s_src,
                                            op=ALU.mult)
                    sq = workp.tile([128, 2 * OUT_NODES], sdt, tag="sq")
                    with nc.allow_low_precision(
                            reason="v1 feeds routing state only"):
                        nc.vector.tensor_reduce(
                            sq[:],
                            ssq[:].rearrange("p g (j d) -> p g j d",
                                             d=OUT_DIM),
                            axis=mybir.AxisListType.X, op=ALU.add,
                        )
                    rt = workp.tile([128, 2 * OUT_NODES], F32, tag="rt")
                    nc.scalar.activation(rt[:], sq[:], AF.Sqrt,
                                         scale=cs * cs)
                    den = workp.tile([128, 2 * OUT_NODES], F32, tag="den")
                    nc.vector.tensor_scalar(den[:], sq[:], cs, 1.0,
                                            ALU.mult, ALU.add)
                    dri = workp.tile([128, 2 * OUT_NODES], F32, tag="dri")
                    nc.vector.reciprocal(dri[:], den[:])
                    f = workp.tile([128, 2 * OUT_NODES], F32, tag="f")
                    nc.vector.tensor_tensor(f[:], rt[:], dri[:],
                                            op=ALU.mult)
                    f_b = (f[:].rearrange("p (g j) -> p g j", j=OUT_NODES)
                           .unsqueeze(3)
                           .broadcast_to([128, 2, OUT_NODES, OUT_DIM]))
                    nc.vector.tensor_tensor(
                        v_sb[:].rearrange("p g (j d) -> p g j d", d=OUT_DIM),
                        s_src.rearrange("p g (j d) -> p g j d", d=OUT_DIM),
                        f_b, op=ALU.mult,
                    )
                    # ---- P = x^T @ v ; Y = reduce_d(W ⊙ P) ; k-sum ;
                    # e_acc *= exp(y) ; c = softmax ; Wc = W ⊙ c ----
                    # The softmax over j is per-(i,k)-row LOCAL, so the
                    # WHOLE b-update chain pipelines per tile-chunk: the
                    # next s-matmul starts after chunk 0's Wc instead of
                    # after the full update. Chunks [0:4)[4:8)[8:9) align
                    # to 2KB PSUM banks (z reads never touch banks
                    # in-flight matmuls write). z/y in bf16 2x DVE mode;
                    # y only feeds routing logits. b logits are never
                    # materialized: e_acc carries the running exp, and Exp
                    # reads the k-sum matmul's PSUM directly.
                    pp_ps = psum.tile([128, NT, JDP], F32, tag="pp_ps",
                                      bufs=1)
                    z_all = workp.tile([128, NT, JD], BF16, tag="z_all")
                    y_all = workp.tile([128, NT * OUT_NODES], BF16,
                                       tag="y_all")
                    y_ps = psum.tile([128, NT * OUT_NODES], F32,
                                     tag="y_ps", bufs=1)
                    ey = workp.tile([128, NT * OUT_NODES], F32, tag="ey")
                    dsum = workp.tile([128, NT], F32, tag="dsum")
                    r = workp.tile([128, NT], F32, tag="r")
                    # c in bf16 so the Wc TTs run all-16-bit in 2x DVE mode
                    c = workp.tile([128, NT * OUT_NODES], BF16, tag="c")
                    r_b = r[:].unsqueeze(2).broadcast_to(
                        [128, NT, OUT_NODES])
                    c_b = (c[:].rearrange("p (t j) -> p t j", j=OUT_NODES)
                           .unsqueeze(3)
                           .broadcast_to([128, NT, OUT_NODES, OUT_DIM]))
                    for lo, hi in ((0, 2), (2, 4), (4, 6), (6, 9)):
                        for t in range(lo, hi):
                            for b0 in range(2):
                                nc.tensor.matmul(
                                    pp_ps[:, t, 0:JD],
                                    x_sb[:, b0 * IK + t * 128:
                                         b0 * IK + t * 128 + 128],
                                    v_sb[:, b0, :],
                                    start=(b0 == 0),
                                    stop=(b0 == 1),
                                )
                        slc = slice(lo, hi)
                        cols = slice(lo * OUT_NODES, hi * OUT_NODES)
                        nc.vector.tensor_tensor(
                            z_all[:, slc, :], WF_sb[:, slc, :],
                            pp_ps[:, slc, 0:JD], op=ALU.mult,
                        )
                        with nc.allow_low_precision(
                                reason="y feeds routing logits only; "
                                       "bf16 d-sum of 16 terms is ample"):
                            nc.vector.tensor_reduce(
                                y_all[:].rearrange("p (t j) -> p t j",
                                                   j=OUT_NODES)[:, slc, :],
                                z_all[:, slc, :].rearrange(
                                    "p t (j d) -> p t j d", d=OUT_DIM),
                                axis=mybir.AxisListType.X, op=ALU.add,
                            )
                        # ones_sb holds 1/B in 8x8 diag blocks: k-sum,
                        # broadcast back over k, and the 1/B in one matmul
                        nc.tensor.matmul(y_ps[:, cols], ones_sb[:],
                                         y_all[:, cols],
                                         start=True, stop=True)
                        nc.scalar.activation(ey[:, cols], y_ps[:, cols],
                                             AF.Exp)
                        nc.vector.tensor_tensor(e_acc[:, cols],
                                                e_acc[:, cols],
                                                ey[:, cols], op=ALU.mult)
                        nc.vector.tensor_reduce(
                            dsum[:, lo:hi],
                            e_acc[:].rearrange("p (t j) -> p t j",
                                               j=OUT_NODES)[:, slc, :],
                            axis=mybir.AxisListType.X, op=ALU.add,
                        )
                        nc.vector.reciprocal(r[:, lo:hi], dsum[:, lo:hi])
                        nc.vector.tensor_tensor(
                            c[:].rearrange("p (t j) -> p t j",
                                           j=OUT_NODES)[:, slc, :],
                            e_acc[:].rearrange("p (t j) -> p t j",
                                               j=OUT_NODES)[:, slc, :],
                            r_b[:, slc], op=ALU.mult,
                        )
                        nc.vector.tensor_tensor(
                            Wc_sb[:, slc, :].rearrange(
                                "p t (j d) -> p t j d", d=OUT_DIM),
                            WF_sb[:, slc, :].rearrange(
                                "p t (j d) -> p t j d", d=OUT_DIM),
                            c_b[:, slc], op=ALU.mult,
                        )

                # ================= iteration 0 (collective-free) ==========
                # s0 = x_full @ W_full computed redundantly on every core
                # from the replicated bf16 tensors; c folding via cs = 0.01.
                # The two batch-half chains are INTERLEAVED tile-by-tile so
                # each xTF/WF tile is consumed for both halves the moment
                # its DMA chunk lands (s0 is input-arrival-paced). That is
                # only legal because the chains sit in SEPARATE PSUM banks
                # (pp tiles 0 and 4 = banks 0 and 2): interleaving two
                # start/stop chains in ONE bank corrupts the accumulation
                # (measured 11% error; separate banks measured exact).
                s0_ps = psum.tile([128, NT, JDP], F32, tag="pp_ps", bufs=1)
                for T in range(NTF):
                    for b0 in range(2):
                        nc.tensor.matmul(
                            s0_ps[:, 4 * b0, 0:JD],
                            xTF_sb[:, T * B + b0 * 128:
                                   T * B + b0 * 128 + 128],
                            WF_sb[:, T, :],
                            start=(T == 0),
                            stop=(T == NTF - 1),
                        )
                # tensor_tensor may read only ONE input from PSUM: copy out
                # cast to bf16 on the way out so iter 0's squash also runs
                # its big ops in 2x DVE mode (v0 is routing-state only)
                s0_sb = workp.tile([128, 2, JD], BF16, tag="s0_sb")
                nc.vector.tensor_copy(s0_sb[:, 0], s0_ps[:, 0, 0:JD])
                nc.vector.tensor_copy(s0_sb[:, 1], s0_ps[:, 4, 0:JD])
                squash_pb_update(s0_sb[:], 0.01, lowp=True)

                # ================= iterations 1..2 ========================
                for it in (1, 2):
                    # s-matmul on the own shard (= rotated tiles [0..NT)):
                    # partial over local i. The two batch-half chains are
                    # interleaved tile-by-tile in SEPARATE PSUM banks
                    # ([2, 512] f32 = one bank per half) so each Wc tile
                    # feeds both halves as soon as its chunk is ready and
                    # both halves finish together - no second 9-matmul
                    # pass on the staging critical path.
                    s_ps = psum.tile([128, 2, 512], F32, tag="s_ps", bufs=1)
                    s_stage = workp.tile([128, 2, JD], BF16, tag="s_stage")
                    sin = dramp.tile([2, 128, JD], BF16, tag="cc_in")
                    for t in range(NT):
                        for b0 in range(2):
                            nc.tensor.matmul(
                                s_ps[:, b0, 0:JD],
                                xTF_sb[:, t * B + b0 * 128:
                                       t * B + b0 * 128 + 128],
                                Wc_sb[:, t, :],
                                start=(t == 0),
                                stop=(t == NT - 1),
                            )
                    for b0 in range(2):
                        nc.vector.tensor_copy(s_stage[:, b0],
                                              s_ps[:, b0, 0:JD])
                        # half-1 stage DMA on gpsimd: same queue as the
                        # collective trigger, avoids a cross-engine hop
                        (nc.sync if b0 == 0 else nc.gpsimd).dma_start(
                            sin[b0], s_stage[:, b0])

                    if it == 1:
                        sout = dramp.tile([2, 128, JD], BF16, tag="cc_out",
                                          addr_space="Shared")
                        nc.gpsimd.collective_compute(
                            "AllReduce", ALU.add, replica_groups=RG,
                            ins=[sin[:]], outs=[sout[:]],
                        )
                        s_sb = workp.tile([128, 2, JD], BF16, tag="s_sb")
                        nc.sync.dma_start(s_sb[:, 0, :], sout[0])
                        nc.scalar.dma_start(s_sb[:, 1, :], sout[1])
                        squash_pb_update(s_sb[:], 1.0, lowp=True)
                    else:
                        # final iter: ReduceScatter; sin flat order is
                        # (g, p, j) so core c owns the 32 consecutive
                        # batches 128*(c//4) + 32*(c%4) + [0,32)
                        sout_rs = dramp.tile([B_LOC, JD], BF16,
                                             tag="cc_out_rs")
                        nc.gpsimd.collective_compute(
                            "ReduceScatter", ALU.add, replica_groups=RG,
                            ins=[sin[:]], outs=[sout_rs[:]],
                        )
                        sl = workp.tile([B_LOC, JD], BF16, tag="sl")
                        nc.sync.dma_start(sl[:], sout_rs[:])
                        # ssq/sq in bf16 2x mode: s is already bf16 off the
                        # wire; the squash scale f tolerates the rounding
                        ssq_l = workp.tile([B_LOC, JD], BF16, tag="ssq_l")
                        nc.vector.tensor_tensor(ssq_l[:], sl[:], sl[:],
                                                op=ALU.mult)
                        sq_l = workp.tile([B_LOC, OUT_NODES], BF16,
                                          tag="sq_l")
                        with nc.allow_low_precision(
                                reason="squash norm; bf16 16-term d-sum"):
                            nc.vector.tensor_reduce(
                                sq_l[:],
                                ssq_l[:].rearrange("p (j d) -> p j d",
                                                   d=OUT_DIM),
                                axis=mybir.AxisListType.X, op=ALU.add,
                            )
                        rt_l = workp.tile([B_LOC, OUT_NODES], F32,
                                          tag="rt_l")
                        nc.scalar.activation(rt_l[:], sq_l[:], AF.Sqrt)
                        den_l = workp.tile([B_LOC, OUT_NODES], F32,
                                           tag="den_l")
                        nc.vector.tensor_scalar_add(den_l[:], sq_l[:], 1.0)
                        dri_l = workp.tile([B_LOC, OUT_NODES], F32,
                                           tag="dri_l")
                        nc.vector.reciprocal(dri_l[:], den_l[:])
                        f_l = workp.tile([B_LOC, OUT_NODES], F32,
                                         tag="f_l")
                        nc.vector.tensor_tensor(f_l[:], rt_l[:], dri_l[:],
                                                op=ALU.mult)
                        v_l = workp.tile([B_LOC, JD], F32, tag="v_l")
                        f_lb = (f_l[:].unsqueeze(2)
                                .broadcast_to([B_LOC, OUT_NODES, OUT_DIM]))
                        nc.vector.tensor_tensor(
                            v_l[:].rearrange("p (j d) -> p j d", d=OUT_DIM),
                            sl[:].rearrange("p (j d) -> p j d", d=OUT_DIM),
                            f_lb, op=ALU.mult,
                        )
                        nc.sync.dma_start(out_d[:], v_l[:])

    nc.compile()
    return nc


def make_inmaps(x, W):
    import ml_dtypes
    bf16 = ml_dtypes.bfloat16
    x = np.ascontiguousarray(np.asarray(x, dtype=np.float32))
    W = np.ascontiguousarray(np.asarray(W, dtype=np.float32))
    x_all = x.reshape(B, IN_NODES * IN_DIM)
    # full tensors as [128, NTF, ...] tile-major views (global tile order)
    xTF_g = np.ascontiguousarray(
        x_all.T.reshape(NTF, 128, B).transpose(1, 0, 2))      # [128,NTF,B]
    wf_g = np.ascontiguousarray(
        W.transpose(0, 3, 1, 2).reshape(NTF, 128, JD).transpose(1, 0, 2))
    # 16 8x8 blocks of 1/B on the diagonal (1/256 is exact in bf16)
    ones_blk = (np.kron(np.eye(128 // IN_DIM, dtype=np.float32),
                        np.ones((IN_DIM, IN_DIM), dtype=np.float32))
                / B).astype(bf16)
    in_maps = []
    for cid in range(N_CORES):
        # rotate tiles so the core's own shard sits at tiles [0..NT)
        rot = np.roll(np.arange(NTF), -cid * NT)
        xTF = np.ascontiguousarray(
            xTF_g[:, rot, :].reshape(128, NTF * B)).astype(bf16)
        wf = np.ascontiguousarray(
            wf_g[:, rot, :].reshape(128, NTF * JD)).astype(bf16)
        sh = slice(cid * I_LOC, (cid + 1) * I_LOC)
        x_sh = x[:, sh, :].reshape(B, IK)
        xb = np.ascontiguousarray(
            x_sh.reshape(2, 128, IK).transpose(1, 0, 2).reshape(
                128, 2 * IK)).astype(bf16)
        in_maps.append({
            "xTF": xTF, "wf": wf, "xb": xb, "onesb": ones_blk,
        })
    return in_maps


def assemble_output(per_core_outs):
    v = np.empty((B, OUT_NODES, OUT_DIM), dtype=np.float32)
    for c in range(N_CORES):
        o = per_core_outs[c]["out"].reshape(B_LOC, OUT_NODES, OUT_DIM)
        st = 128 * (c // 4) + B_LOC * (c % 4)
        v[st:st + B_LOC] = o
    return v[..., None].astype(np.float32)      # (256, 10, 16, 1)


_CACHED_NC = None


def kernel(x=None, W=None, **kw):
    global _CACHED_NC
    if x is None:
        x = kw["x"]
    if W is None:
        W = kw["W"]
    if _CACHED_NC is None:
        _CACHED_NC = build_nc()
    in_maps = make_inmaps(x, W)
    res = run_bass_kernel_spmd(
        _CACHED_NC, in_maps, core_ids=list(range(N_CORES)))
    return assemble_output(res.results)


if __name__ == "__main__":
    nc = build_nc()
    print("build + compile OK")
